# revision 11
# baseline (speedup 1.0000x reference)
"""Trainium2 Bass kernel: ComplexGabor1D layer.

reference math (fp32):
    lin = x @ W.T + b                      # [N, 256]
    env = exp(-3600 * lin^2)
    out = stack([env*cos(30*lin), env*sin(30*lin)], -1)   # [N, 256, 2]

Strategy (8 NeuronCores, data parallel over N):
  * Host: transpose each x shard to [256, N_SH] so the contraction dim (i)
    lands on SBUF partitions with fully-contiguous DMA loads; replicate
    W.T ([in, out]) and b on every core.
  * Device, per 512-row "unit": fp32r matmuls (x_shard.T tiles as the
    stationary operand, W.T as the moving operand, bias added via a K=1
    rank-1 matmul) -> lin in PSUM; ACT computes sin/cos straight from PSUM
    into the interleaved output tile (real at even, imag at odd offsets);
    square on ACT or DVE (split to balance engines); ACT exp; DVE multiplies
    the envelope into both strided halves in place; DMA out 1 MiB per unit.
  * ACT activation tables: sin and exp live in different table sets
    (~2.7us per switch), so units are processed in groups: all trig work
    for a group first, then all exp work -> 2 switches per group.
  * cos(t) is computed as sin(t + pi/2).  The argument exceeds the Sin
    LUT's [-pi, pi] window only where |30*lin| > pi/2, i.e. where the
    Gaussian envelope is < 5.2e-5, so the hardware clamp there is
    numerically invisible at the output (abs err <= ~1e-4 of absmax 1.0).
"""

import math

import numpy as np

import concourse.bacc as bacc
import concourse.bass as bass
import concourse.mybir as mybir
import concourse.tile as tile
from concourse.bass_utils import run_bass_kernel_spmd

N_TOTAL = 262144
IN_F = 256
OUT_F = 256
N_CORES = 8
N_SH = N_TOTAL // N_CORES  # 32768 rows per core

CHUNK = 128  # rows per matmul (PSUM partition dim)
CH_PER_UNIT = 4  # chunks per unit -> 512 rows, F=1024 elementwise ops
GROUP_UNITS = 9  # units per ACT-table-set group

OMEGA = 30.0
NEG_SCALE2 = -3600.0  # -(60^2)

F32 = mybir.dt.float32
F32R = mybir.dt.float32r
BF16 = mybir.dt.bfloat16

_BUILD_CACHE = {}


def _build(n_sh, ch_per_unit, group_units):
    """Build the single-core Bass program (SPMD across cores via in_maps)."""
    key = (n_sh, ch_per_unit, group_units)
    if key in _BUILD_CACHE:
        return _BUILD_CACHE[key]

    rows_per_unit = CHUNK * ch_per_unit
    assert n_sh % rows_per_unit == 0
    n_units = n_sh // rows_per_unit

    nc = bacc.Bacc("TRN2", target_bir_lowering=False, debug=False)

    xt = nc.dram_tensor("xt", [IN_F, n_sh], F32R, kind="ExternalInput").ap()
    wt = nc.dram_tensor("wt", [IN_F, OUT_F], F32R, kind="ExternalInput").ap()
    bias = nc.dram_tensor(
        "bias", [CHUNK, ch_per_unit * OUT_F], F32, kind="ExternalInput"
    ).ap()
    out = nc.dram_tensor("out", [n_sh, 2 * OUT_F], F32, kind="ExternalOutput").ap()

    # [i, n] -> [p, ci, n] with i = ci*128 + p
    xt_r = xt.rearrange("(ci p) n -> p ci n", p=CHUNK)
    # [i, o] -> [p, ci, o]
    wt_r = wt.rearrange("(ci p) o -> p ci o", p=CHUNK)
    # row n = u*rows_per_unit + c*128 + p
    out_r = out.rearrange("(u c p) f -> u p c f", p=CHUNK, c=ch_per_unit)

    with tile.TileContext(nc) as tc:
        with (
            tc.tile_pool(name="consts", bufs=1) as consts,
            tc.tile_pool(name="xt", bufs=3) as xt_pool,
            tc.tile_pool(name="linsb", bufs=group_units + 1) as linsb_pool,
            tc.tile_pool(name="sq", bufs=group_units + 1) as sq_pool,
            tc.tile_pool(name="outp", bufs=group_units + 1) as out_pool,
            tc.tile_pool(name="lin", bufs=4, space="PSUM") as psum_pool,
        ):
            wt_sb = consts.tile([CHUNK, IN_F // CHUNK, OUT_F], F32R)
            nc.sync.dma_start(wt_sb[:], wt_r[:])
            # bias broadcast across all 128 partitions, tiled x4 along free
            b_sb = consts.tile([CHUNK, ch_per_unit, OUT_F], F32)
            nc.sync.dma_start(
                b_sb[:], bias.rearrange("p (c o) -> p c o", c=ch_per_unit)
            )
            zero_b = consts.tile([CHUNK, 1], F32)
            nc.vector.memset(zero_b[:], 0.0)
            pio2_b = consts.tile([CHUNK, 1], F32)
            nc.vector.memset(pio2_b[:], math.pi / 2)

            prev_act = [None]

            def act_chain(inst):
                # Pin the ACT engine's instruction order to emission order so
                # the scheduler cannot interleave exp into the sin stream
                # (each such jump costs two ~1.3us ACT table loads).
                if prev_act[0] is not None:
                    tile.add_dep_helper(inst.ins, prev_act[0], sync=False,
                                        reason="act table-set order")
                prev_act[0] = inst.ins

            xt_carry = None
            n_groups = (n_units + group_units - 1) // group_units
            for g in range(n_groups):
                units = range(g * group_units, min((g + 1) * group_units, n_units))
                staged = []

                # ---- trig phase (sin table set resident) ----
                for u in units:
                    n0 = u * rows_per_unit
                    if u % 2 == 0 and u + 1 < n_units:
                        # one 1 MiB load covering two units: 4 KiB runs/partition
                        xt_t2 = xt_pool.tile(
                            [CHUNK, IN_F // CHUNK, 2 * rows_per_unit], F32R,
                            tag="xt2",
                        )
                        nc.sync.dma_start(
                            xt_t2[:], xt_r[:, :, n0 : n0 + 2 * rows_per_unit]
                        )
                        xt_carry = xt_t2
                        xt_t, xt_off = xt_t2, 0
                    elif u % 2 == 1 and xt_carry is not None:
                        xt_t, xt_off = xt_carry, rows_per_unit
                    else:
                        xt_t = xt_pool.tile(
                            [CHUNK, IN_F // CHUNK, rows_per_unit], F32R, tag="xt1"
                        )
                        nc.sync.dma_start(
                            xt_t[:], xt_r[:, :, n0 : n0 + rows_per_unit]
                        )
                        xt_off = 0

                    lin = psum_pool.tile([CHUNK, ch_per_unit, OUT_F], F32)
                    for c in range(ch_per_unit):
                        j0 = xt_off + c * CHUNK
                        nc.tensor.matmul(
                            lin[:, c, :],
                            xt_t[:, 0, j0 : j0 + CHUNK],
                            wt_sb[:, 0, :],
                            start=True,
                            stop=False,
                        )
                        nc.tensor.matmul(
                            lin[:, c, :],
                            xt_t[:, 1, j0 : j0 + CHUNK],
                            wt_sb[:, 1, :],
                            start=False,
                            stop=True,
                        )

                    # Drain PSUM via an unchained DVE copy so the PE is never
                    # gated on the ACT table-set phase order.
                    lin_sb = linsb_pool.tile([CHUNK, ch_per_unit, OUT_F], F32)
                    nc.vector.scalar_tensor_tensor(
                        lin_sb[:],
                        lin[:],
                        1.0,
                        b_sb[:],
                        op0=mybir.AluOpType.mult,
                        op1=mybir.AluOpType.add,
                    )

                    out_t = out_pool.tile([CHUNK, ch_per_unit, 2 * OUT_F], F32)
                    out4 = out_t[:].rearrange("p c (o two) -> p c o two", two=2)
                    # imag = sin(30*lin), real = cos = sin(30*lin + pi/2)
                    act_chain(nc.scalar.activation(
                        out4[:, :, :, 1],
                        lin_sb[:],
                        mybir.ActivationFunctionType.Sin,
                        bias=zero_b[:],
                        scale=OMEGA,
                    ))
                    act_chain(nc.scalar.activation(
                        out4[:, :, :, 0],
                        lin_sb[:],
                        mybir.ActivationFunctionType.Sin,
                        bias=pio2_b[:],
                        scale=OMEGA,
                    ))
                    sq = sq_pool.tile([CHUNK, ch_per_unit, OUT_F], F32)
                    nc.gpsimd.tensor_mul(sq[:], lin_sb[:], lin_sb[:])
                    staged.append((u, out_t, sq))

                # ---- exp phase (exp table set resident) ----
                for u, out_t, sq in staged:
                    act_chain(nc.scalar.activation(
                        sq[:],
                        sq[:],
                        mybir.ActivationFunctionType.Exp,
                        bias=zero_b[:],
                        scale=NEG_SCALE2,
                    ))
                    out4 = out_t[:].rearrange("p c (o two) -> p c o two", two=2)
                    mul_eng = nc.gpsimd if u % 10 == 0 else nc.vector
                    mul_eng.tensor_mul(out4[:, :, :, 0], out4[:, :, :, 0], sq[:])
                    mul_eng.tensor_mul(out4[:, :, :, 1], out4[:, :, :, 1], sq[:])
                    # SWDGE so output stores don't head-of-line block input loads
                    nc.gpsimd.dma_start(out_r[u], out_t[:])

    nc.compile()
    _BUILD_CACHE[key] = nc
    return nc


def run_sharded(x, W, b, trace=False, n_sh=N_SH, ch_per_unit=CH_PER_UNIT,
                group_units=GROUP_UNITS):
    """Shard inputs over the 8 cores, run the Bass kernel, gather output."""
    x = np.ascontiguousarray(x, dtype=np.float32)
    W = np.ascontiguousarray(W, dtype=np.float32)
    b = np.ascontiguousarray(b, dtype=np.float32)
    n = x.shape[0]
    assert n == n_sh * N_CORES and x.shape[1] == IN_F

    nc = _build(n_sh, ch_per_unit, group_units)

    wt_np = np.ascontiguousarray(W.T)  # [in, out]
    b_np = np.ascontiguousarray(
        np.broadcast_to(
            np.tile(b, ch_per_unit)[None, :], (CHUNK, ch_per_unit * OUT_F)
        )
    )
    in_maps = []
    for s in range(N_CORES):
        xt_np = np.ascontiguousarray(x[s * n_sh : (s + 1) * n_sh].T)  # [in, n_sh]
        in_maps.append({"xt": xt_np, "wt": wt_np, "bias": b_np})

    res = run_bass_kernel_spmd(nc, in_maps, list(range(N_CORES)), trace=trace)
    shards = [
        res.results[s]["out"].reshape(n_sh, OUT_F, 2) for s in range(N_CORES)
    ]
    return np.concatenate(shards, axis=0), res


def kernel(x, W, b):
    out, _ = run_sharded(x, W, b)
    return out


# revision 12
# speedup vs baseline: 1.0074x; 1.0074x over previous
"""Trainium2 Bass kernel: ComplexGabor1D layer.

reference math (fp32):
    lin = x @ W.T + b                      # [N, 256]
    env = exp(-3600 * lin^2)
    out = stack([env*cos(30*lin), env*sin(30*lin)], -1)   # [N, 256, 2]

Strategy (8 NeuronCores, data parallel over N):
  * Host: transpose each x shard to [256, N_SH] so the contraction dim (i)
    lands on SBUF partitions with fully-contiguous DMA loads; replicate
    W.T ([in, out]) and b on every core.
  * Device, per 512-row "unit": fp32r matmuls (x_shard.T tiles as the
    stationary operand, W.T as the moving operand, bias added via a K=1
    rank-1 matmul) -> lin in PSUM; ACT computes sin/cos straight from PSUM
    into the interleaved output tile (real at even, imag at odd offsets);
    square on ACT or DVE (split to balance engines); ACT exp; DVE multiplies
    the envelope into both strided halves in place; DMA out 1 MiB per unit.
  * ACT activation tables: sin and exp live in different table sets
    (~2.7us per switch), so units are processed in groups: all trig work
    for a group first, then all exp work -> 2 switches per group.
  * cos(t) is computed as sin(t + pi/2).  The argument exceeds the Sin
    LUT's [-pi, pi] window only where |30*lin| > pi/2, i.e. where the
    Gaussian envelope is < 5.2e-5, so the hardware clamp there is
    numerically invisible at the output (abs err <= ~1e-4 of absmax 1.0).
"""

import math

import numpy as np

import concourse.bacc as bacc
import concourse.bass as bass
import concourse.mybir as mybir
import concourse.tile as tile
from concourse.bass_utils import run_bass_kernel_spmd

N_TOTAL = 262144
IN_F = 256
OUT_F = 256
N_CORES = 8
N_SH = N_TOTAL // N_CORES  # 32768 rows per core

CHUNK = 128  # rows per matmul (PSUM partition dim)
CH_PER_UNIT = 4  # chunks per unit -> 512 rows, F=1024 elementwise ops
GROUP_UNITS = 12  # units per ACT-table-set group

OMEGA = 30.0
NEG_SCALE2 = -3600.0  # -(60^2)

F32 = mybir.dt.float32
F32R = mybir.dt.float32r
BF16 = mybir.dt.bfloat16

_BUILD_CACHE = {}


def _build(n_sh, ch_per_unit, group_units):
    """Build the single-core Bass program (SPMD across cores via in_maps)."""
    key = (n_sh, ch_per_unit, group_units)
    if key in _BUILD_CACHE:
        return _BUILD_CACHE[key]

    rows_per_unit = CHUNK * ch_per_unit
    assert n_sh % rows_per_unit == 0
    n_units = n_sh // rows_per_unit

    nc = bacc.Bacc("TRN2", target_bir_lowering=False, debug=False)

    xt = nc.dram_tensor("xt", [IN_F, n_sh], F32R, kind="ExternalInput").ap()
    wt = nc.dram_tensor("wt", [IN_F, OUT_F], F32R, kind="ExternalInput").ap()
    bias = nc.dram_tensor(
        "bias", [CHUNK, ch_per_unit * OUT_F], F32, kind="ExternalInput"
    ).ap()
    out = nc.dram_tensor("out", [n_sh, 2 * OUT_F], F32, kind="ExternalOutput").ap()

    # [i, n] -> [p, ci, n] with i = ci*128 + p
    xt_r = xt.rearrange("(ci p) n -> p ci n", p=CHUNK)
    # [i, o] -> [p, ci, o]
    wt_r = wt.rearrange("(ci p) o -> p ci o", p=CHUNK)
    # row n = u*rows_per_unit + c*128 + p
    out_r = out.rearrange("(u c p) f -> u p c f", p=CHUNK, c=ch_per_unit)

    with tile.TileContext(nc) as tc:
        with (
            tc.tile_pool(name="consts", bufs=1) as consts,
            tc.tile_pool(name="xt", bufs=3) as xt_pool,
            tc.tile_pool(name="linsb", bufs=group_units + 1) as linsb_pool,
            tc.tile_pool(name="outp", bufs=group_units + 1) as out_pool,
            tc.tile_pool(name="lin", bufs=4, space="PSUM") as psum_pool,
        ):
            wt_sb = consts.tile([CHUNK, IN_F // CHUNK, OUT_F], F32R)
            nc.sync.dma_start(wt_sb[:], wt_r[:])
            # bias broadcast across all 128 partitions, tiled x4 along free
            b_sb = consts.tile([CHUNK, ch_per_unit, OUT_F], F32)
            nc.sync.dma_start(
                b_sb[:], bias.rearrange("p (c o) -> p c o", c=ch_per_unit)
            )
            zero_b = consts.tile([CHUNK, 1], F32)
            nc.vector.memset(zero_b[:], 0.0)
            pio2_b = consts.tile([CHUNK, 1], F32)
            nc.vector.memset(pio2_b[:], math.pi / 2)

            prev_act = [None]

            def act_chain(inst):
                # Pin the ACT engine's instruction order to emission order so
                # the scheduler cannot interleave exp into the sin stream
                # (each such jump costs two ~1.3us ACT table loads).
                if prev_act[0] is not None:
                    tile.add_dep_helper(inst.ins, prev_act[0], sync=False,
                                        reason="act table-set order")
                prev_act[0] = inst.ins

            xt_carry = None
            n_groups = (n_units + group_units - 1) // group_units
            for g in range(n_groups):
                units = range(g * group_units, min((g + 1) * group_units, n_units))
                staged = []

                # ---- trig phase (sin table set resident) ----
                for u in units:
                    n0 = u * rows_per_unit
                    if u % 2 == 0 and u + 1 < n_units:
                        # one 1 MiB load covering two units: 4 KiB runs/partition
                        xt_t2 = xt_pool.tile(
                            [CHUNK, IN_F // CHUNK, 2 * rows_per_unit], F32R,
                            tag="xt2",
                        )
                        nc.sync.dma_start(
                            xt_t2[:], xt_r[:, :, n0 : n0 + 2 * rows_per_unit]
                        )
                        xt_carry = xt_t2
                        xt_t, xt_off = xt_t2, 0
                    elif u % 2 == 1 and xt_carry is not None:
                        xt_t, xt_off = xt_carry, rows_per_unit
                    else:
                        xt_t = xt_pool.tile(
                            [CHUNK, IN_F // CHUNK, rows_per_unit], F32R, tag="xt1"
                        )
                        nc.sync.dma_start(
                            xt_t[:], xt_r[:, :, n0 : n0 + rows_per_unit]
                        )
                        xt_off = 0

                    lin = psum_pool.tile([CHUNK, ch_per_unit, OUT_F], F32)
                    for c in range(ch_per_unit):
                        j0 = xt_off + c * CHUNK
                        nc.tensor.matmul(
                            lin[:, c, :],
                            xt_t[:, 0, j0 : j0 + CHUNK],
                            wt_sb[:, 0, :],
                            start=True,
                            stop=False,
                        )
                        nc.tensor.matmul(
                            lin[:, c, :],
                            xt_t[:, 1, j0 : j0 + CHUNK],
                            wt_sb[:, 1, :],
                            start=False,
                            stop=True,
                        )

                    # Drain PSUM via an unchained DVE copy so the PE is never
                    # gated on the ACT table-set phase order.
                    lin_sb = linsb_pool.tile([CHUNK, ch_per_unit, OUT_F], F32)
                    nc.vector.scalar_tensor_tensor(
                        lin_sb[:],
                        lin[:],
                        1.0,
                        b_sb[:],
                        op0=mybir.AluOpType.mult,
                        op1=mybir.AluOpType.add,
                    )

                    out_t = out_pool.tile([CHUNK, ch_per_unit, 2 * OUT_F], F32)
                    out4 = out_t[:].rearrange("p c (o two) -> p c o two", two=2)
                    # imag = sin(30*lin), real = cos = sin(30*lin + pi/2)
                    act_chain(nc.scalar.activation(
                        out4[:, :, :, 1],
                        lin_sb[:],
                        mybir.ActivationFunctionType.Sin,
                        bias=zero_b[:],
                        scale=OMEGA,
                    ))
                    act_chain(nc.scalar.activation(
                        out4[:, :, :, 0],
                        lin_sb[:],
                        mybir.ActivationFunctionType.Sin,
                        bias=pio2_b[:],
                        scale=OMEGA,
                    ))
                    # square lin_sb in place; the WAR dep on sin/cos orders it
                    # after both trig reads, still within this group's phase.
                    if u % 16 < 7:
                        act_chain(nc.scalar.activation(
                            lin_sb[:],
                            lin_sb[:],
                            mybir.ActivationFunctionType.Square,
                            bias=zero_b[:],
                            scale=1.0,
                        ))
                    else:
                        nc.vector.tensor_mul(lin_sb[:], lin_sb[:], lin_sb[:])
                    staged.append((u, out_t, lin_sb))

                # ---- exp phase (exp table set resident) ----
                for u, out_t, env in staged:
                    act_chain(nc.scalar.activation(
                        env[:],
                        env[:],
                        mybir.ActivationFunctionType.Exp,
                        bias=zero_b[:],
                        scale=NEG_SCALE2,
                    ))
                    out4 = out_t[:].rearrange("p c (o two) -> p c o two", two=2)
                    nc.vector.tensor_mul(out4[:, :, :, 0], out4[:, :, :, 0], env[:])
                    nc.vector.tensor_mul(out4[:, :, :, 1], out4[:, :, :, 1], env[:])
                    # SWDGE so output stores don't head-of-line block input loads
                    nc.gpsimd.dma_start(out_r[u], out_t[:])

    nc.compile()
    _BUILD_CACHE[key] = nc
    return nc


def run_sharded(x, W, b, trace=False, n_sh=N_SH, ch_per_unit=CH_PER_UNIT,
                group_units=GROUP_UNITS):
    """Shard inputs over the 8 cores, run the Bass kernel, gather output."""
    x = np.ascontiguousarray(x, dtype=np.float32)
    W = np.ascontiguousarray(W, dtype=np.float32)
    b = np.ascontiguousarray(b, dtype=np.float32)
    n = x.shape[0]
    assert n == n_sh * N_CORES and x.shape[1] == IN_F

    nc = _build(n_sh, ch_per_unit, group_units)

    wt_np = np.ascontiguousarray(W.T)  # [in, out]
    b_np = np.ascontiguousarray(
        np.broadcast_to(
            np.tile(b, ch_per_unit)[None, :], (CHUNK, ch_per_unit * OUT_F)
        )
    )
    in_maps = []
    for s in range(N_CORES):
        xt_np = np.ascontiguousarray(x[s * n_sh : (s + 1) * n_sh].T)  # [in, n_sh]
        in_maps.append({"xt": xt_np, "wt": wt_np, "bias": b_np})

    res = run_bass_kernel_spmd(nc, in_maps, list(range(N_CORES)), trace=trace)
    shards = [
        res.results[s]["out"].reshape(n_sh, OUT_F, 2) for s in range(N_CORES)
    ]
    return np.concatenate(shards, axis=0), res


def kernel(x, W, b):
    out, _ = run_sharded(x, W, b)
    return out


# revision 13
# speedup vs baseline: 1.0845x; 1.0766x over previous
"""Trainium2 Bass kernel: ComplexGabor1D layer.

reference math (fp32):
    lin = x @ W.T + b                      # [N, 256]
    env = exp(-3600 * lin^2)
    out = stack([env*cos(30*lin), env*sin(30*lin)], -1)   # [N, 256, 2]

Strategy (8 NeuronCores, data parallel over N):
  * Host: transpose each x shard to [256, N_SH] so the contraction dim (i)
    lands on SBUF partitions with fully-contiguous DMA loads; replicate
    W.T ([in, out]) and b on every core.
  * Device, per 512-row "unit": fp32r matmuls (x_shard.T tiles as the
    stationary operand, W.T as the moving operand, bias added via a K=1
    rank-1 matmul) -> lin in PSUM; ACT computes sin/cos straight from PSUM
    into the interleaved output tile (real at even, imag at odd offsets);
    square on ACT or DVE (split to balance engines); ACT exp; DVE multiplies
    the envelope into both strided halves in place; DMA out 1 MiB per unit.
  * ACT activation tables: sin and exp live in different table sets
    (~2.7us per switch), so units are processed in groups: all trig work
    for a group first, then all exp work -> 2 switches per group.
  * cos(t) is computed as sin(t + pi/2).  The argument exceeds the Sin
    LUT's [-pi, pi] window only where |30*lin| > pi/2, i.e. where the
    Gaussian envelope is < 5.2e-5, so the hardware clamp there is
    numerically invisible at the output (abs err <= ~1e-4 of absmax 1.0).
"""

import math

import numpy as np

import concourse.bacc as bacc
import concourse.bass as bass
import concourse.mybir as mybir
import concourse.tile as tile
from concourse.bass_utils import run_bass_kernel_spmd

N_TOTAL = 262144
IN_F = 256
OUT_F = 256
N_CORES = 8
N_SH = N_TOTAL // N_CORES  # 32768 rows per core

CHUNK = 128  # rows per matmul (PSUM partition dim)
CH_PER_UNIT = 4  # chunks per unit -> 512 rows, F=1024 elementwise ops
GROUP_UNITS = 12  # units per ACT-table-set group

OMEGA = 30.0
NEG_SCALE2 = -3600.0  # -(60^2)

F32 = mybir.dt.float32
F32R = mybir.dt.float32r
BF16 = mybir.dt.bfloat16

_BUILD_CACHE = {}


def _build(n_sh, ch_per_unit, group_units):
    """Build the single-core Bass program (SPMD across cores via in_maps)."""
    key = (n_sh, ch_per_unit, group_units)
    if key in _BUILD_CACHE:
        return _BUILD_CACHE[key]

    rows_per_unit = CHUNK * ch_per_unit
    assert n_sh % rows_per_unit == 0
    n_units = n_sh // rows_per_unit

    nc = bacc.Bacc("TRN2", target_bir_lowering=False, debug=False)

    xt = nc.dram_tensor("xt", [IN_F, n_sh], F32R, kind="ExternalInput").ap()
    wt = nc.dram_tensor("wt", [IN_F, OUT_F], F32R, kind="ExternalInput").ap()
    bias = nc.dram_tensor(
        "bias", [CHUNK, ch_per_unit * OUT_F], F32, kind="ExternalInput"
    ).ap()
    out = nc.dram_tensor("out", [n_sh, 2 * OUT_F], F32, kind="ExternalOutput").ap()

    # [i, n] -> [p, ci, n] with i = ci*128 + p
    xt_r = xt.rearrange("(ci p) n -> p ci n", p=CHUNK)
    # [i, o] -> [p, ci, o]
    wt_r = wt.rearrange("(ci p) o -> p ci o", p=CHUNK)
    # row n = u*rows_per_unit + c*128 + p
    out_r = out.rearrange("(u c p) f -> u p c f", p=CHUNK, c=ch_per_unit)

    with tile.TileContext(nc) as tc:
        with (
            tc.tile_pool(name="consts", bufs=1) as consts,
            tc.tile_pool(name="xt", bufs=3) as xt_pool,
            tc.tile_pool(name="linsb", bufs=group_units + 1) as linsb_pool,
            tc.tile_pool(name="outp", bufs=group_units + 1) as out_pool,
            tc.tile_pool(name="lin", bufs=4, space="PSUM") as psum_pool,
        ):
            wt_sb = consts.tile([CHUNK, IN_F // CHUNK, OUT_F], F32R)
            nc.sync.dma_start(wt_sb[:], wt_r[:])
            # bias broadcast across all 128 partitions, tiled x4 along free
            b_sb = consts.tile([CHUNK, ch_per_unit, OUT_F], F32)
            nc.sync.dma_start(
                b_sb[:], bias.rearrange("p (c o) -> p c o", c=ch_per_unit)
            )
            zero_b = consts.tile([CHUNK, 1], F32)
            nc.vector.memset(zero_b[:], 0.0)
            pio2_b = consts.tile([CHUNK, 1], F32)
            nc.vector.memset(pio2_b[:], math.pi / 2)

            prev_act = [None]

            def act_chain(inst):
                # Pin the ACT engine's instruction order to emission order so
                # the scheduler cannot interleave exp into the sin stream
                # (each such jump costs two ~1.3us ACT table loads).
                if prev_act[0] is not None:
                    tile.add_dep_helper(inst.ins, prev_act[0], sync=False,
                                        reason="act table-set order")
                prev_act[0] = inst.ins

            xt_carry = None
            n_groups = (n_units + group_units - 1) // group_units
            for g in range(n_groups):
                units = range(g * group_units, min((g + 1) * group_units, n_units))
                staged = []

                # ---- trig phase (sin table set resident) ----
                for u in units:
                    n0 = u * rows_per_unit
                    if u % 2 == 0 and u + 1 < n_units:
                        # one 1 MiB load covering two units: 4 KiB runs/partition
                        xt_t2 = xt_pool.tile(
                            [CHUNK, IN_F // CHUNK, 2 * rows_per_unit], F32R,
                            tag="xt2",
                        )
                        nc.sync.dma_start(
                            xt_t2[:], xt_r[:, :, n0 : n0 + 2 * rows_per_unit]
                        )
                        xt_carry = xt_t2
                        xt_t, xt_off = xt_t2, 0
                    elif u % 2 == 1 and xt_carry is not None:
                        xt_t, xt_off = xt_carry, rows_per_unit
                    else:
                        xt_t = xt_pool.tile(
                            [CHUNK, IN_F // CHUNK, rows_per_unit], F32R, tag="xt1"
                        )
                        nc.sync.dma_start(
                            xt_t[:], xt_r[:, :, n0 : n0 + rows_per_unit]
                        )
                        xt_off = 0

                    lin = psum_pool.tile([CHUNK, ch_per_unit, OUT_F], F32)
                    for c in range(ch_per_unit):
                        j0 = xt_off + c * CHUNK
                        nc.tensor.matmul(
                            lin[:, c, :],
                            xt_t[:, 0, j0 : j0 + CHUNK],
                            wt_sb[:, 0, :],
                            start=True,
                            stop=False,
                        )
                        nc.tensor.matmul(
                            lin[:, c, :],
                            xt_t[:, 1, j0 : j0 + CHUNK],
                            wt_sb[:, 1, :],
                            start=False,
                            stop=True,
                        )

                    # Drain PSUM via an unchained DVE copy so the PE is never
                    # gated on the ACT table-set phase order.
                    lin_sb = linsb_pool.tile([CHUNK, ch_per_unit, OUT_F], F32)
                    nc.vector.scalar_tensor_tensor(
                        lin_sb[:],
                        lin[:],
                        1.0,
                        b_sb[:],
                        op0=mybir.AluOpType.mult,
                        op1=mybir.AluOpType.add,
                    )

                    out_t = out_pool.tile([CHUNK, ch_per_unit, 2 * OUT_F], F32)
                    out4 = out_t[:].rearrange("p c (o two) -> p c o two", two=2)
                    # imag = sin(30*lin), real = cos = sin(30*lin + pi/2)
                    act_chain(nc.scalar.activation(
                        out4[:, :, :, 1],
                        lin_sb[:],
                        mybir.ActivationFunctionType.Sin,
                        bias=zero_b[:],
                        scale=OMEGA,
                    ))
                    act_chain(nc.scalar.activation(
                        out4[:, :, :, 0],
                        lin_sb[:],
                        mybir.ActivationFunctionType.Sin,
                        bias=pio2_b[:],
                        scale=OMEGA,
                    ))
                    staged.append((u, out_t, lin_sb))

                # ---- exp phase (exp table set resident) ----
                for u, out_t, env in staged:
                    act_chain(nc.scalar.activation(
                        env[:],
                        env[:],
                        mybir.ActivationFunctionType.Square,
                        bias=zero_b[:],
                        scale=1.0,
                    ))
                    act_chain(nc.scalar.activation(
                        env[:],
                        env[:],
                        mybir.ActivationFunctionType.Exp,
                        bias=zero_b[:],
                        scale=NEG_SCALE2,
                    ))
                    out4 = out_t[:].rearrange("p c (o two) -> p c o two", two=2)
                    nc.vector.tensor_mul(out4[:, :, :, 0], out4[:, :, :, 0], env[:])
                    nc.vector.tensor_mul(out4[:, :, :, 1], out4[:, :, :, 1], env[:])
                    # SWDGE so output stores don't head-of-line block input loads
                    nc.gpsimd.dma_start(out_r[u], out_t[:])

    nc.compile()
    _BUILD_CACHE[key] = nc
    return nc


def run_sharded(x, W, b, trace=False, n_sh=N_SH, ch_per_unit=CH_PER_UNIT,
                group_units=GROUP_UNITS):
    """Shard inputs over the 8 cores, run the Bass kernel, gather output."""
    x = np.ascontiguousarray(x, dtype=np.float32)
    W = np.ascontiguousarray(W, dtype=np.float32)
    b = np.ascontiguousarray(b, dtype=np.float32)
    n = x.shape[0]
    assert n == n_sh * N_CORES and x.shape[1] == IN_F

    nc = _build(n_sh, ch_per_unit, group_units)

    wt_np = np.ascontiguousarray(W.T)  # [in, out]
    b_np = np.ascontiguousarray(
        np.broadcast_to(
            np.tile(b, ch_per_unit)[None, :], (CHUNK, ch_per_unit * OUT_F)
        )
    )
    in_maps = []
    for s in range(N_CORES):
        xt_np = np.ascontiguousarray(x[s * n_sh : (s + 1) * n_sh].T)  # [in, n_sh]
        in_maps.append({"xt": xt_np, "wt": wt_np, "bias": b_np})

    res = run_bass_kernel_spmd(nc, in_maps, list(range(N_CORES)), trace=trace)
    shards = [
        res.results[s]["out"].reshape(n_sh, OUT_F, 2) for s in range(N_CORES)
    ]
    return np.concatenate(shards, axis=0), res


def kernel(x, W, b):
    out, _ = run_sharded(x, W, b)
    return out


# revision 14
# speedup vs baseline: 1.0963x; 1.0108x over previous
"""Trainium2 Bass kernel: ComplexGabor1D layer.

reference math (fp32):
    lin = x @ W.T + b                      # [N, 256]
    env = exp(-3600 * lin^2)
    out = stack([env*cos(30*lin), env*sin(30*lin)], -1)   # [N, 256, 2]

Strategy (8 NeuronCores, data parallel over N):
  * Host: transpose each x shard to [256, N_SH] so the contraction dim (i)
    lands on SBUF partitions with fully-contiguous DMA loads; replicate
    W.T ([in, out]) and the bias (pre-broadcast to 128 partitions).
  * Device, per 1024-row "pair" (2 x 512-row halves, 8 x 128-row chunks):
    fp32r matmuls (x.T tiles stationary, W.T moving) accumulate lin into
    PSUM; a fused DVE scalar_tensor_tensor drains PSUM to SBUF while adding
    the bias (lin_sb = lin*1 + b) so the PE is never gated on ACT phases;
    ACT writes sin/cos straight into the interleaved output tile (real at
    even, imag at odd offsets); the envelope is squared+exp'ed in place on
    lin_sb; DVE multiplies the envelope into both strided halves in place;
    2 MiB output DMA per pair via SWDGE so stores don't block input loads.
  * ACT activation tables: sin and exp live in different table sets
    (~2.7us per switch), so pairs are processed in groups: all trig work
    for a group first, then all envelope work -> 2 switches per group. The
    ACT instruction order is pinned via dep edges to stop the scheduler
    interleaving exp into the sin stream.  A fraction of the squares runs
    on DVE (emitted first, their exps last) to balance ACT vs DVE.
  * cos(t) is computed as sin(t + pi/2).  The argument exceeds the Sin
    LUT's [-pi, pi] window only where |30*lin| > pi/2, i.e. where the
    Gaussian envelope is < 5.2e-5, so the hardware clamp there is
    numerically invisible at the output (abs err <= ~1e-4 of absmax 1.0).
"""

import math

import numpy as np

import concourse.bacc as bacc
import concourse.mybir as mybir
import concourse.tile as tile
from concourse.bass_utils import run_bass_kernel_spmd

N_TOTAL = 262144
IN_F = 256
OUT_F = 256
N_CORES = 8
N_SH = N_TOTAL // N_CORES  # 32768 rows per core

CHUNK = 128  # rows per matmul (PSUM partition dim)
CH_PER_HALF = 4  # chunks per half-pair -> 512 rows
HALVES = 2  # halves per pair -> 1024 rows, F=2048 elementwise ops
GROUP_PAIRS = 5  # pairs per ACT-table-set group

OMEGA = 30.0
NEG_SCALE2 = -3600.0  # -(60^2)

F32 = mybir.dt.float32
F32R = mybir.dt.float32r

_BUILD_CACHE = {}


def _build(n_sh, group_pairs):
    """Build the single-core Bass program (SPMD across cores via in_maps)."""
    key = (n_sh, group_pairs)
    if key in _BUILD_CACHE:
        return _BUILD_CACHE[key]

    rows_per_half = CHUNK * CH_PER_HALF
    rows_per_pair = rows_per_half * HALVES
    assert n_sh % rows_per_pair == 0
    n_pairs = n_sh // rows_per_pair

    nc = bacc.Bacc("TRN2", target_bir_lowering=False, debug=False)

    xt = nc.dram_tensor("xt", [IN_F, n_sh], F32R, kind="ExternalInput").ap()
    wt = nc.dram_tensor("wt", [IN_F, OUT_F], F32R, kind="ExternalInput").ap()
    bias = nc.dram_tensor(
        "bias", [CHUNK, CH_PER_HALF * OUT_F], F32, kind="ExternalInput"
    ).ap()
    out = nc.dram_tensor("out", [n_sh, 2 * OUT_F], F32, kind="ExternalOutput").ap()

    # [i, n] -> [p, ci, n] with i = ci*128 + p
    xt_r = xt.rearrange("(ci p) n -> p ci n", p=CHUNK)
    wt_r = wt.rearrange("(ci p) o -> p ci o", p=CHUNK)
    # row n = pr*1024 + t*512 + c*128 + p
    out_r = out.rearrange(
        "(pr t c p) f -> pr p t c f", p=CHUNK, c=CH_PER_HALF, t=HALVES
    )

    with tile.TileContext(nc) as tc:
        with (
            tc.tile_pool(name="consts", bufs=1) as consts,
            tc.tile_pool(name="xt", bufs=3) as xt_pool,
            tc.tile_pool(name="linsb", bufs=group_pairs + 1) as linsb_pool,
            tc.tile_pool(name="outp", bufs=group_pairs + 1) as out_pool,
            tc.tile_pool(name="lin", bufs=4, space="PSUM") as psum_pool,
        ):
            wt_sb = consts.tile([CHUNK, IN_F // CHUNK, OUT_F], F32R)
            nc.sync.dma_start(wt_sb[:], wt_r[:])
            b_sb = consts.tile([CHUNK, CH_PER_HALF, OUT_F], F32)
            nc.sync.dma_start(
                b_sb[:], bias.rearrange("p (c o) -> p c o", c=CH_PER_HALF)
            )
            zero_b = consts.tile([CHUNK, 1], F32)
            nc.vector.memset(zero_b[:], 0.0)
            pio2_b = consts.tile([CHUNK, 1], F32)
            nc.vector.memset(pio2_b[:], math.pi / 2)

            prev_act = [None]

            def act_chain(inst):
                # Pin the ACT engine's instruction order to emission order so
                # the scheduler cannot interleave exp into the sin stream
                # (each such jump costs two ~1.3us ACT table loads).
                if prev_act[0] is not None:
                    tile.add_dep_helper(inst.ins, prev_act[0], sync=False,
                                        reason="act table-set order")
                prev_act[0] = inst.ins

            n_groups = (n_pairs + group_pairs - 1) // group_pairs
            for g in range(n_groups):
                pairs = range(g * group_pairs, min((g + 1) * group_pairs, n_pairs))
                staged = []

                # ---- trig phase (sin table set resident) ----
                for pr in pairs:
                    n0 = pr * rows_per_pair
                    # one 1 MiB load covering the pair: 4 KiB runs/partition
                    xt_t = xt_pool.tile([CHUNK, IN_F // CHUNK, rows_per_pair], F32R)
                    nc.sync.dma_start(xt_t[:], xt_r[:, :, n0 : n0 + rows_per_pair])

                    lin_sb = linsb_pool.tile(
                        [CHUNK, HALVES, CH_PER_HALF, OUT_F], F32
                    )
                    for t in range(HALVES):
                        lin = psum_pool.tile([CHUNK, CH_PER_HALF, OUT_F], F32)
                        for c in range(CH_PER_HALF):
                            j0 = t * rows_per_half + c * CHUNK
                            nc.tensor.matmul(
                                lin[:, c, :],
                                xt_t[:, 0, j0 : j0 + CHUNK],
                                wt_sb[:, 0, :],
                                start=True,
                                stop=False,
                            )
                            nc.tensor.matmul(
                                lin[:, c, :],
                                xt_t[:, 1, j0 : j0 + CHUNK],
                                wt_sb[:, 1, :],
                                start=False,
                                stop=True,
                            )
                        # drain PSUM with a fused bias add: lin_sb = lin + b
                        nc.vector.scalar_tensor_tensor(
                            lin_sb[:, t, :, :],
                            lin[:],
                            1.0,
                            b_sb[:],
                            op0=mybir.AluOpType.mult,
                            op1=mybir.AluOpType.add,
                        )

                    out_t = out_pool.tile(
                        [CHUNK, HALVES, CH_PER_HALF, 2 * OUT_F], F32
                    )
                    out5 = out_t[:].rearrange(
                        "p t c (o two) -> p t c o two", two=2
                    )
                    # imag = sin(30*lin), real = cos = sin(30*lin + pi/2)
                    act_chain(nc.scalar.activation(
                        out5[:, :, :, :, 1],
                        lin_sb[:],
                        mybir.ActivationFunctionType.Sin,
                        bias=zero_b[:],
                        scale=OMEGA,
                    ))
                    act_chain(nc.scalar.activation(
                        out5[:, :, :, :, 0],
                        lin_sb[:],
                        mybir.ActivationFunctionType.Sin,
                        bias=pio2_b[:],
                        scale=OMEGA,
                    ))
                    staged.append((pr, out_t, lin_sb))

                # ---- envelope phase (exp table set resident) ----
                # ~30% of squares go to DVE: emitted first, their exps last,
                # so ACT never waits on a just-in-time DVE square.
                dve_sq = [s for s in staged if s[0] % 10 in (2, 5, 8)]
                act_sq = [s for s in staged if s[0] % 10 not in (2, 5, 8)]
                for pr, out_t, env in dve_sq:
                    nc.vector.tensor_mul(env[:], env[:], env[:])
                for with_act_square, group_part in ((True, act_sq), (False, dve_sq)):
                    for pr, out_t, env in group_part:
                        if with_act_square:
                            act_chain(nc.scalar.activation(
                                env[:],
                                env[:],
                                mybir.ActivationFunctionType.Square,
                                bias=zero_b[:],
                                scale=1.0,
                            ))
                        act_chain(nc.scalar.activation(
                            env[:],
                            env[:],
                            mybir.ActivationFunctionType.Exp,
                            bias=zero_b[:],
                            scale=NEG_SCALE2,
                        ))
                        out5 = out_t[:].rearrange(
                            "p t c (o two) -> p t c o two", two=2
                        )
                        nc.vector.tensor_mul(
                            out5[:, :, :, :, 0], out5[:, :, :, :, 0], env[:]
                        )
                        nc.vector.tensor_mul(
                            out5[:, :, :, :, 1], out5[:, :, :, :, 1], env[:]
                        )
                        # SWDGE so output stores don't head-of-line block loads
                        nc.gpsimd.dma_start(out_r[pr], out_t[:])

    nc.compile()
    _BUILD_CACHE[key] = nc
    return nc


def run_sharded(x, W, b, trace=False, n_sh=N_SH, group_pairs=GROUP_PAIRS):
    """Shard inputs over the 8 cores, run the Bass kernel, gather output."""
    x = np.ascontiguousarray(x, dtype=np.float32)
    W = np.ascontiguousarray(W, dtype=np.float32)
    b = np.ascontiguousarray(b, dtype=np.float32)
    n = x.shape[0]
    assert n == n_sh * N_CORES and x.shape[1] == IN_F

    nc = _build(n_sh, group_pairs)

    wt_np = np.ascontiguousarray(W.T)  # [in, out]
    b_np = np.ascontiguousarray(
        np.broadcast_to(
            np.tile(b, CH_PER_HALF)[None, :], (CHUNK, CH_PER_HALF * OUT_F)
        )
    )
    in_maps = []
    for s in range(N_CORES):
        xt_np = np.ascontiguousarray(x[s * n_sh : (s + 1) * n_sh].T)  # [in, n_sh]
        in_maps.append({"xt": xt_np, "wt": wt_np, "bias": b_np})

    res = run_bass_kernel_spmd(nc, in_maps, list(range(N_CORES)), trace=trace)
    shards = [
        res.results[s]["out"].reshape(n_sh, OUT_F, 2) for s in range(N_CORES)
    ]
    return np.concatenate(shards, axis=0), res


def kernel(x, W, b):
    out, _ = run_sharded(x, W, b)
    return out
